# revision 5
# baseline (speedup 1.0000x reference)
"""Single-head attention (Q/K/V proj + softmax(QK^T)V) on 8 trn2 NeuronCores, v3.

Data-parallel over batch B=16 -> 2 batches/core, zero communication.

Math restructure vs the v1 baseline: softmax is invariant to per-query
additive constants, so with A = Wq Wk^T and w = x (Wk bq),
    softmax((xWq+bq)(xWk+bk)^T) = softmax(g x^T + 1 w^T),  g = x A.
The k-projection, bk load, and q/k bias passes disappear; w folds into the
exp as a per-partition activation bias. A and wkbq are computed on device
once (~9us of PE) from Wq/Wk/bq.

Layout: x is uploaded channel-major ([PB, D, N]) and Wq/Wk transposed
([e, d]), all host-marshalled, so the
d-on-partitions x^T that every matmul wants is a plain chunk DMA -- no PE
transposes, no PSUM->SBUF x copies. Both batches' x^T prefetch fully into
SBUF during batch 0.

Per core, per batch:
  phase A  per 512-token window: v = x Wv + bv, token-major
           (xT-stationary matmuls, bias-add on DVE).
  A-setup  (once): wkbq^T row-matmuls + tiny transposes -> wkbq_p, then
           A = Wq Wk^T straight from the uploaded e-major weights.
  w-rows   per window: w^T[1,512] = wkbq^T-contract of xT; tiny transposes
           land w_j on j-partitions -> wT[128, NT].
  phase B  per 512-query block: g chunks (A-stationary, xT moving), S^T
           tiles (xT-stationary, g moving), exp(S^T + w_j) straight to SBUF
           f32r; DVE pre-reduces the 16 P^T tiles, ones-matmul + tiny
           transposes + reciprocal give 1/rowsum; PV accumulates over 16
           j-tiles with the normalization folded into the PSUM->SBUF scale;
           out-DMA dispatched from scalar right after each scale.

Queues: sync + gpsimd split the xT chunk DMAs (2 queues keep the v-window
pace); scalar carries Wv/Wk/Wq then the output DMAs (fast drain at exit,
unlike gpsimd whose tail drain costs ~4us).

Measured (clean thermal state): ~350.7-351.4us vs 366.9us for the v1
baseline;
rel err 7.86e-4. PE busy 329.5us at the f32r floor (steady-state 512-row
matmul issue interval 234ns, LDWEIGHTS fully hidden); phase A is at the
~330GB/s per-core HBM read cap; head 7.1us prologue + ~3us first-DMA
latency; tail ~5us Tile exit barrier. Note: back-to-back benches trigger
DVFS throttling (~20% slower, NTFF throttle_avg_util_limit ~0.92).
"""

import os

import numpy as np

try:
    from antenv.axon_hooks import get_axon_ntff_profile_hook  # noqa: F401
except ImportError:
    os.environ.setdefault("BASS_NEVER_TRACE", "1")

import concourse.bass as bass
import concourse.tile as tile
from concourse import bacc, mybir
from concourse.bass_utils import run_bass_kernel_spmd
from concourse.masks import make_identity

f32 = mybir.dt.float32
f32r = mybir.dt.float32r

B, N, D = 16, 2048, 512
NCORES = 8
PB = B // NCORES
NT = N // 128
DC = D // 128
NIB = N // 512
NW = N // 512
JT = NT


def build():
    nc = bacc.Bacc("TRN2", target_bir_lowering=False, debug=False)

    xt = nc.dram_tensor("xt", [PB, D, N], f32, kind="ExternalInput")
    # WqT/WkT uploaded already transposed ([e, d] layout, host-marshalled):
    # the A = Wq Wk^T product contracts over e, so both operands want e on
    # partitions — this deletes all 32 A-setup PE transposes.
    WqT = nc.dram_tensor("WqT", [D, D], f32, kind="ExternalInput")
    bq = nc.dram_tensor("bq", [D], f32, kind="ExternalInput")
    WkT = nc.dram_tensor("WkT", [D, D], f32, kind="ExternalInput")
    Wv = nc.dram_tensor("Wv", [D, D], f32, kind="ExternalInput")
    bv = nc.dram_tensor("bv", [D], f32, kind="ExternalInput")
    out = nc.dram_tensor("out", [PB, N, D], f32, kind="ExternalOutput")

    with tile.TileContext(nc) as tc:
        with (
            tc.tile_pool(name="singles", bufs=1) as singles,
            tc.tile_pool(name="wpool", bufs=1) as wpool,
            tc.tile_pool(name="xTp", bufs=1) as xTp,
            tc.tile_pool(name="vvp", bufs=1) as vvp,
            tc.tile_pool(name="wrow", bufs=2) as wrow,
            tc.tile_pool(name="spool", bufs=1) as spool,
            tc.tile_pool(name="rpool", bufs=1) as rpool,
            tc.tile_pool(name="psbank", bufs=5, space="PSUM") as psbank,
            tc.tile_pool(name="pspv", bufs=2, space="PSUM") as pspv,
            tc.tile_pool(name="pstrans", bufs=1, space="PSUM") as pstrans,
        ):
            # biases first (tiny; the first v-adds need bv_bc ~+12us)
            bv_bc = singles.tile([128, D], f32)
            bv_ap = bv[:]
            nc.gpsimd.dma_start(
                out=bv_bc[:],
                in_=bass.AP(
                    tensor=bv_ap.tensor, offset=bv_ap.offset, ap=[[0, 128], *bv_ap.ap]
                ),
            )
            bqT = singles.tile([128, DC], f32r)
            nc.gpsimd.dma_start(
                out=bqT[:], in_=bq[:].rearrange("(dc p) -> p dc", p=128).bitcast(f32r)
            )

            # Wv on scalar, halved chunks, h-outer: the first v-projection's
            # h0 accumulation chain can start once the 4 h0 pieces land.
            wv_r = wpool.tile([128, DC, D], f32r, tag="wv")
            for h in range(2):
                csl = slice(h * 256, (h + 1) * 256)
                for dc in range(DC):
                    nc.scalar.dma_start(
                        out=wv_r[:, dc, csl],
                        in_=Wv[dc * 128 : (dc + 1) * 128, csl].bitcast(f32r),
                    )
            A_r = wpool.tile([128, DC, D], f32r, tag="A")

            xTs = []
            for b in range(PB):
                xT = xTp.tile([128, DC, N], f32r, tag=f"xT{b}")
                xTs.append(xT)

            def xt_dma(b, w, pieces, dc3_eng=None):
                """xT chunk DMAs for one window: dc 0/1 sync, dc 2/3 gpsimd
                (dc3 optionally rerouted, e.g. to scalar once Wv is through)."""
                for dc in range(DC):
                    eng = nc.sync if dc < 2 else nc.gpsimd
                    if dc == 3 and dc3_eng is not None:
                        eng = dc3_eng
                    for p in range(pieces):
                        psl = slice(
                            w * 512 + p * (512 // pieces),
                            w * 512 + (p + 1) * (512 // pieces),
                        )
                        eng.dma_start(
                            out=xTs[b][:, dc, psl],
                            in_=xt[b, dc * 128 : (dc + 1) * 128, psl].bitcast(f32r),
                        )

            # b0: window 0 at jt-tile granularity (low first-chunk latency),
            # rest window-granular with dc3 on scalar (lands behind Wv there,
            # relieving the gpsimd queue which paces v(w1..w3)).
            xt_dma(0, 0, 4)
            for w in range(1, NW):
                xt_dma(0, w, 1, dc3_eng=nc.scalar)
            # WqT split across sync+gpsimd (A-setup needs it ~+34us; scalar is
            # busy with Wv+WkT until ~+31us). ec = e-chunk of the [e, d] layout.
            wqT_w = wpool.tile([128, DC, D], f32r, tag="wq")
            for ec in range(DC):
                eng = nc.sync if ec < 2 else nc.gpsimd
                eng.dma_start(
                    out=wqT_w[:, ec, :],
                    in_=WqT[ec * 128 : (ec + 1) * 128, :].bitcast(f32r),
                )
            # b1 prefetch behind everything critical
            for w in range(NW):
                xt_dma(1, w, 1)

            ident = singles.tile([128, 128], f32)
            make_identity(nc, ident[:])
            ones_f32 = singles.tile([128, 1], f32)
            nc.vector.memset(ones_f32[:], 1.0)
            ones_r_t = singles.tile([128, 1], f32r)
            nc.vector.tensor_copy(ones_r_t[:], ones_f32[:])
            ones_r = ones_r_t[:]

            wkbq_p = singles.tile([128, DC], f32r)
            wT = singles.tile([128, NT], f32)

            vv = vvp.tile([128, NT, D], f32r)

            def v_window(xT, w, halves=False):
                for jt in range(w * 4, w * 4 + 4):
                    jsl = slice(jt * 128, (jt + 1) * 128)
                    pv = psbank.tile([128, 512], f32, tag="bank")
                    if halves:
                        # two sequential column accumulation chains (they must
                        # NOT interleave: start/stop chains corrupt each other
                        # when alternating within one PSUM bank)
                        for h in range(2):
                            csl = slice(h * 256, (h + 1) * 256)
                            for dc in range(DC):
                                nc.tensor.matmul(
                                    pv[:, csl],
                                    xT[:, dc, jsl],
                                    wv_r[:, dc, csl],
                                    start=(dc == 0),
                                    stop=(dc == DC - 1),
                                )
                    else:
                        for dc in range(DC):
                            nc.tensor.matmul(
                                pv[:],
                                xT[:, dc, jsl],
                                wv_r[:, dc, :],
                                start=(dc == 0),
                                stop=(dc == DC - 1),
                            )
                    nc.vector.tensor_add(vv[:, jt, :], pv[:], bv_bc[:])

            def w_rows(xT, w):
                pw = psbank.tile([1, 512], f32, tag="bank")
                for dc in range(DC):
                    nc.tensor.matmul(
                        pw[:],
                        wkbq_p[:, dc : dc + 1],
                        xT[:, dc, w * 512 : (w + 1) * 512],
                        start=(dc == 0),
                        stop=(dc == DC - 1),
                    )
                w_sb = wrow.tile([1, 512], f32, tag="wsb")
                nc.vector.tensor_copy(w_sb[:], pw[:])
                st = pstrans.tile([128, 4], f32, tag="st")
                for c in range(4):
                    nc.tensor.transpose(
                        st[:, c : c + 1],
                        w_sb[0:1, c * 128 : (c + 1) * 128],
                        ident[0:1, 0:1],
                    )
                nc.vector.tensor_copy(wT[:, 4 * w : 4 * w + 4], st[:])

            # ---- batch 0 phase A
            v_window(xTs[0], 0, halves=True)
            for w in range(1, NW):
                v_window(xTs[0], w)

            # ---- A-setup (WkT on scalar behind Wv; no transposes needed —
            # both weights arrive already e-major)
            with tc.tile_pool(name="wtrans", bufs=1) as wtrans:
                wkT = wtrans.tile([128, DC, D], f32r, tag="wkT")
                for ec in range(DC):
                    nc.scalar.dma_start(
                        out=wkT[:, ec, :],
                        in_=WkT[ec * 128 : (ec + 1) * 128, :].bitcast(f32r),
                    )
                wqT = wqT_w
                # wkbq^T[1, d] = sum_e bq[e] Wk[d, e]
                pw = psbank.tile([1, 512], f32, tag="bank")
                for ec in range(DC):
                    nc.tensor.matmul(
                        pw[:],
                        bqT[:, ec : ec + 1],
                        wkT[:, ec, :],
                        start=(ec == 0),
                        stop=(ec == DC - 1),
                    )
                wk_sb = wrow.tile([1, 512], f32, tag="wsb")
                nc.vector.tensor_copy(wk_sb[:], pw[:])
                st = pstrans.tile([128, 4], f32, tag="st")
                for c in range(4):
                    nc.tensor.transpose(
                        st[:, c : c + 1],
                        wk_sb[0:1, c * 128 : (c + 1) * 128],
                        ident[0:1, 0:1],
                    )
                nc.vector.tensor_copy(wkbq_p[:], st[:])
                # A[d, d'] = sum_e Wq[d, e] Wk[d', e]
                for dc in range(DC):
                    dsl = slice(dc * 128, (dc + 1) * 128)
                    pa = psbank.tile([128, 512], f32, tag="bank")
                    for ec in range(DC):
                        nc.tensor.matmul(
                            pa[:],
                            wqT[:, ec, dsl],
                            wkT[:, ec, :],
                            start=(ec == 0),
                            stop=(ec == DC - 1),
                        )
                    nc.scalar.copy(A_r[:, dc, :], pa[:])

            with (
                tc.tile_pool(name="pTp", bufs=1) as pTp,
                tc.tile_pool(name="gtp", bufs=2) as gtp,
                tc.tile_pool(name="redp", bufs=1) as redp,
                tc.tile_pool(name="ostage", bufs=2) as ostage,
            ):
                pT = pTp.tile([128, JT, 512], f32r)

                def attention(b):
                    xT = xTs[b]
                    for ib in range(NIB):
                        isl = slice(ib * 512, (ib + 1) * 512)
                        gt = gtp.tile([128, DC, 512], f32r)
                        for ec in range(DC):
                            esl = slice(ec * 128, (ec + 1) * 128)
                            pg = psbank.tile([128, 512], f32, tag="bank")
                            for dc in range(DC):
                                nc.tensor.matmul(
                                    pg[:],
                                    A_r[:, dc, esl],
                                    xT[:, dc, isl],
                                    start=(dc == 0),
                                    stop=(dc == DC - 1),
                                )
                            nc.scalar.copy(gt[:, ec, :], pg[:])
                        for jt in range(JT):
                            jsl = slice(jt * 128, (jt + 1) * 128)
                            ps = psbank.tile([128, 512], f32, tag="bank")
                            for c in range(DC):
                                nc.tensor.matmul(
                                    ps[:],
                                    xT[:, c, jsl],
                                    gt[:, c, :],
                                    start=(c == 0),
                                    stop=(c == DC - 1),
                                )
                            nc.scalar.activation(
                                pT[:, jt, :],
                                ps[:],
                                mybir.ActivationFunctionType.Exp,
                                bias=wT[:, jt : jt + 1],
                            )
                        red = redp.tile([128, 2, 512], f32r, tag="red")
                        for g in range(2):
                            nc.vector.tensor_add(
                                red[:, g, :], pT[:, 8 * g, :], pT[:, 8 * g + 1, :]
                            )
                            for j in range(8 * g + 2, 8 * g + 8):
                                nc.vector.tensor_add(
                                    red[:, g, :], red[:, g, :], pT[:, j, :]
                                )
                        nc.vector.tensor_add(red[:, 0, :], red[:, 0, :], red[:, 1, :])
                        sums_p = psbank.tile([1, 512], f32, tag="bank")
                        nc.tensor.matmul(
                            sums_p[:], ones_r, red[:, 0, :], start=True, stop=True
                        )
                        s_sb = spool.tile([1, 512], f32)
                        nc.vector.tensor_copy(s_sb[:], sums_p[:])
                        st_p = pstrans.tile([128, 4], f32, tag="st")
                        for c in range(4):
                            nc.tensor.transpose(
                                st_p[:, c : c + 1],
                                s_sb[0:1, c * 128 : (c + 1) * 128],
                                ident[0:1, 0:1],
                            )
                        r_sb = rpool.tile([128, 4], f32, tag="r")
                        nc.vector.reciprocal(r_sb[:], st_p[:])
                        for isub in range(4):
                            po = pspv.tile([128, 512], f32)
                            for jt in range(JT):
                                nc.tensor.matmul(
                                    po[:],
                                    pT[:, jt, isub * 128 : (isub + 1) * 128],
                                    vv[:, jt, :],
                                    start=(jt == 0),
                                    stop=(jt == JT - 1),
                                )
                            ob = ostage.tile([128, 512], f32, tag="ob")
                            t0 = ib * 512 + isub * 128
                            nc.scalar.mul(ob[:], po[:], r_sb[:, isub : isub + 1])
                            nc.scalar.dma_start(out=out[b, t0 : t0 + 128, :], in_=ob[:])

                for w in range(NW):
                    w_rows(xTs[0], w)
                attention(0)

                for w in range(NW):
                    v_window(xTs[1], w)
                for w in range(NW):
                    w_rows(xTs[1], w)
                attention(1)

    nc.finalize()
    return nc


_built = None


def kernel(x, Wq, bq, Wk, bk, Wv, bv):
    global _built
    x = np.asarray(x, dtype=np.float32)
    xt = np.ascontiguousarray(np.swapaxes(x, 1, 2))  # [B, D, N]
    ws = {
        "WqT": np.ascontiguousarray(np.asarray(Wq, dtype=np.float32).T),
        "bq": np.ascontiguousarray(np.asarray(bq, dtype=np.float32)),
        "WkT": np.ascontiguousarray(np.asarray(Wk, dtype=np.float32).T),
        "Wv": np.ascontiguousarray(np.asarray(Wv, dtype=np.float32)),
        "bv": np.ascontiguousarray(np.asarray(bv, dtype=np.float32)),
    }
    if _built is None:
        _built = build()
    in_maps = [
        {"xt": np.ascontiguousarray(xt[c * PB : (c + 1) * PB]), **ws}
        for c in range(NCORES)
    ]
    res = run_bass_kernel_spmd(_built, in_maps, core_ids=list(range(NCORES)))
    kernel.last_exec_time_ns = res.exec_time_ns
    return np.concatenate([r["out"] for r in res.results], axis=0)


kernel.last_exec_time_ns = None


# revision 6
# speedup vs baseline: 1.0024x; 1.0024x over previous
"""Single-head attention (Q/K/V proj + softmax(QK^T)V) on 8 trn2 NeuronCores, v3.

Data-parallel over batch B=16 -> 2 batches/core, zero communication.

Math restructure vs the v1 baseline: softmax is invariant to per-query
additive constants, so with A = Wq Wk^T and w = x (Wk bq),
    softmax((xWq+bq)(xWk+bk)^T) = softmax(g x^T + 1 w^T),  g = x A.
The k-projection, bk load, and q/k bias passes disappear; w folds into the
exp as a per-partition activation bias. A and wkbq are computed on device
once (~9us of PE) from Wq/Wk/bq.

Layout: x is uploaded channel-major ([PB, D, N]) and Wq/Wk transposed
([e, d]), all host-marshalled, so the
d-on-partitions x^T that every matmul wants is a plain chunk DMA -- no PE
transposes, no PSUM->SBUF x copies. Both batches' x^T prefetch fully into
SBUF during batch 0.

Per core, per batch:
  phase A  per 512-token window: v = x Wv + bv, token-major
           (xT-stationary matmuls, bias-add on DVE).
  A-setup  (once): wkbq^T row-matmuls + tiny transposes -> wkbq_p, then
           A = Wq Wk^T straight from the uploaded e-major weights.
  w-rows   per window: w^T[1,512] = wkbq^T-contract of xT; tiny transposes
           land w_j on j-partitions -> wT[128, NT].
  phase B  per 512-query block: g chunks (A-stationary, xT moving), S^T
           tiles (xT-stationary, g moving), exp(S^T + w_j) straight to SBUF
           f32r; DVE pre-reduces the 16 P^T tiles, ones-matmul + tiny
           transposes + reciprocal give 1/rowsum; PV accumulates over 16
           j-tiles with the normalization folded into the PSUM->SBUF scale;
           out-DMA dispatched from scalar right after each scale.

Queues: sync + gpsimd split the xT chunk DMAs (2 queues keep the v-window
pace); scalar carries Wv/Wk/Wq then the output DMAs (fast drain at exit,
unlike gpsimd whose tail drain costs ~4us).

Measured (clean thermal state): ~350.7-351.4us vs 366.9us for the v1
baseline;
rel err 7.86e-4. PE busy 329.5us at the f32r floor (steady-state 512-row
matmul issue interval 234ns, LDWEIGHTS fully hidden); phase A is at the
~330GB/s per-core HBM read cap; head 7.1us prologue + ~3us first-DMA
latency; tail ~5us Tile exit barrier. Note: back-to-back benches trigger
DVFS throttling (~20% slower, NTFF throttle_avg_util_limit ~0.92).
"""

import os

import numpy as np

try:
    from antenv.axon_hooks import get_axon_ntff_profile_hook  # noqa: F401
except ImportError:
    os.environ.setdefault("BASS_NEVER_TRACE", "1")

import concourse.bass as bass
import concourse.tile as tile
from concourse import bacc, mybir
from concourse.bass_utils import run_bass_kernel_spmd
from concourse.masks import make_identity

f32 = mybir.dt.float32
f32r = mybir.dt.float32r

B, N, D = 16, 2048, 512
NCORES = 8
PB = B // NCORES
NT = N // 128
DC = D // 128
NIB = N // 512
NW = N // 512
JT = NT


def build():
    nc = bacc.Bacc("TRN2", target_bir_lowering=False, debug=False)

    xt = nc.dram_tensor("xt", [PB, D, N], f32, kind="ExternalInput")
    # WqT/WkT uploaded already transposed ([e, d] layout, host-marshalled):
    # the A = Wq Wk^T product contracts over e, so both operands want e on
    # partitions — this deletes all 32 A-setup PE transposes.
    WqT = nc.dram_tensor("WqT", [D, D], f32, kind="ExternalInput")
    bq = nc.dram_tensor("bq", [D], f32, kind="ExternalInput")
    WkT = nc.dram_tensor("WkT", [D, D], f32, kind="ExternalInput")
    Wv = nc.dram_tensor("Wv", [D, D], f32, kind="ExternalInput")
    bv = nc.dram_tensor("bv", [D], f32, kind="ExternalInput")
    out = nc.dram_tensor("out", [PB, N, D], f32, kind="ExternalOutput")

    with tile.TileContext(nc) as tc:
        with (
            tc.tile_pool(name="singles", bufs=1) as singles,
            tc.tile_pool(name="wpool", bufs=1) as wpool,
            tc.tile_pool(name="xTp", bufs=1) as xTp,
            tc.tile_pool(name="vvp", bufs=1) as vvp,
            tc.tile_pool(name="wrow", bufs=2) as wrow,
            tc.tile_pool(name="spool", bufs=1) as spool,
            tc.tile_pool(name="rpool", bufs=1) as rpool,
            tc.tile_pool(name="psbank", bufs=5, space="PSUM") as psbank,
            tc.tile_pool(name="pspv", bufs=2, space="PSUM") as pspv,
            tc.tile_pool(name="pstrans", bufs=1, space="PSUM") as pstrans,
        ):
            # biases first (tiny; the first v-adds need bv_bc ~+12us)
            bv_bc = singles.tile([128, D], f32)
            bv_ap = bv[:]
            nc.gpsimd.dma_start(
                out=bv_bc[:],
                in_=bass.AP(
                    tensor=bv_ap.tensor, offset=bv_ap.offset, ap=[[0, 128], *bv_ap.ap]
                ),
            )
            bqT = singles.tile([128, DC], f32r)
            nc.gpsimd.dma_start(
                out=bqT[:], in_=bq[:].rearrange("(dc p) -> p dc", p=128).bitcast(f32r)
            )

            # Wv on scalar, halved chunks, h-outer: the first v-projection's
            # h0 accumulation chain can start once the 4 h0 pieces land.
            wv_r = wpool.tile([128, DC, D], f32r, tag="wv")
            for h in range(2):
                csl = slice(h * 256, (h + 1) * 256)
                for dc in range(DC):
                    nc.scalar.dma_start(
                        out=wv_r[:, dc, csl],
                        in_=Wv[dc * 128 : (dc + 1) * 128, csl].bitcast(f32r),
                    )
            A_r = wpool.tile([128, DC, D], f32r, tag="A")

            xTs = []
            for b in range(PB):
                xT = xTp.tile([128, DC, N], f32r, tag=f"xT{b}")
                xTs.append(xT)

            def xt_dma(b, w, pieces, dc3_eng=None):
                """xT chunk DMAs for one window: dc 0/1 sync, dc 2/3 gpsimd
                (dc3 optionally rerouted, e.g. to scalar once Wv is through)."""
                for dc in range(DC):
                    eng = nc.sync if dc < 2 else nc.gpsimd
                    if dc == 3 and dc3_eng is not None:
                        eng = dc3_eng
                    for p in range(pieces):
                        psl = slice(
                            w * 512 + p * (512 // pieces),
                            w * 512 + (p + 1) * (512 // pieces),
                        )
                        eng.dma_start(
                            out=xTs[b][:, dc, psl],
                            in_=xt[b, dc * 128 : (dc + 1) * 128, psl].bitcast(f32r),
                        )

            # b0: window 0 at jt-tile granularity (low first-chunk latency),
            # rest window-granular with dc3 on scalar (lands behind Wv there,
            # relieving the gpsimd queue which paces v(w1..w3)).
            xt_dma(0, 0, 4)
            for w in range(1, NW):
                xt_dma(0, w, 1, dc3_eng=nc.scalar)
            # WkT split across sync+gpsimd (~+30us): wkbq is the FIRST
            # A-setup consumer, so WkT must beat WqT (which rides scalar
            # behind Wv+dc3 and is only needed ~1us later by the A matmuls).
            wkT_w = wpool.tile([128, DC, D], f32r, tag="wk")
            for ec in range(DC):
                eng = nc.sync if ec < 2 else nc.gpsimd
                eng.dma_start(
                    out=wkT_w[:, ec, :],
                    in_=WkT[ec * 128 : (ec + 1) * 128, :].bitcast(f32r),
                )
            # b1 prefetch behind everything critical
            for w in range(NW):
                xt_dma(1, w, 1)

            ident = singles.tile([128, 128], f32)
            make_identity(nc, ident[:])
            ones_f32 = singles.tile([128, 1], f32)
            nc.vector.memset(ones_f32[:], 1.0)
            ones_r_t = singles.tile([128, 1], f32r)
            nc.vector.tensor_copy(ones_r_t[:], ones_f32[:])
            ones_r = ones_r_t[:]

            wkbq_p = singles.tile([128, DC], f32r)
            wT = singles.tile([128, NT], f32)

            vv = vvp.tile([128, NT, D], f32r)

            def v_window(xT, w, halves=False):
                for jt in range(w * 4, w * 4 + 4):
                    jsl = slice(jt * 128, (jt + 1) * 128)
                    pv = psbank.tile([128, 512], f32, tag="bank")
                    if halves:
                        # two sequential column accumulation chains (they must
                        # NOT interleave: start/stop chains corrupt each other
                        # when alternating within one PSUM bank)
                        for h in range(2):
                            csl = slice(h * 256, (h + 1) * 256)
                            for dc in range(DC):
                                nc.tensor.matmul(
                                    pv[:, csl],
                                    xT[:, dc, jsl],
                                    wv_r[:, dc, csl],
                                    start=(dc == 0),
                                    stop=(dc == DC - 1),
                                )
                    else:
                        for dc in range(DC):
                            nc.tensor.matmul(
                                pv[:],
                                xT[:, dc, jsl],
                                wv_r[:, dc, :],
                                start=(dc == 0),
                                stop=(dc == DC - 1),
                            )
                    nc.vector.tensor_add(vv[:, jt, :], pv[:], bv_bc[:])

            def w_rows(xT, w):
                pw = psbank.tile([1, 512], f32, tag="bank")
                for dc in range(DC):
                    nc.tensor.matmul(
                        pw[:],
                        wkbq_p[:, dc : dc + 1],
                        xT[:, dc, w * 512 : (w + 1) * 512],
                        start=(dc == 0),
                        stop=(dc == DC - 1),
                    )
                w_sb = wrow.tile([1, 512], f32, tag="wsb")
                nc.vector.tensor_copy(w_sb[:], pw[:])
                st = pstrans.tile([128, 4], f32, tag="st")
                for c in range(4):
                    nc.tensor.transpose(
                        st[:, c : c + 1],
                        w_sb[0:1, c * 128 : (c + 1) * 128],
                        ident[0:1, 0:1],
                    )
                nc.vector.tensor_copy(wT[:, 4 * w : 4 * w + 4], st[:])

            # ---- batch 0 phase A
            v_window(xTs[0], 0, halves=True)
            for w in range(1, NW):
                v_window(xTs[0], w)

            # ---- A-setup (WqT on scalar behind Wv; no transposes needed —
            # both weights arrive already e-major)
            with tc.tile_pool(name="wtrans", bufs=1) as wtrans:
                wqT = wtrans.tile([128, DC, D], f32r, tag="wqT")
                for ec in range(DC):
                    nc.scalar.dma_start(
                        out=wqT[:, ec, :],
                        in_=WqT[ec * 128 : (ec + 1) * 128, :].bitcast(f32r),
                    )
                wkT = wkT_w
                # wkbq^T[1, d] = sum_e bq[e] Wk[d, e]
                pw = psbank.tile([1, 512], f32, tag="bank")
                for ec in range(DC):
                    nc.tensor.matmul(
                        pw[:],
                        bqT[:, ec : ec + 1],
                        wkT[:, ec, :],
                        start=(ec == 0),
                        stop=(ec == DC - 1),
                    )
                wk_sb = wrow.tile([1, 512], f32, tag="wsb")
                nc.vector.tensor_copy(wk_sb[:], pw[:])
                st = pstrans.tile([128, 4], f32, tag="st")
                for c in range(4):
                    nc.tensor.transpose(
                        st[:, c : c + 1],
                        wk_sb[0:1, c * 128 : (c + 1) * 128],
                        ident[0:1, 0:1],
                    )
                nc.vector.tensor_copy(wkbq_p[:], st[:])
                # A[d, d'] = sum_e Wq[d, e] Wk[d', e]
                for dc in range(DC):
                    dsl = slice(dc * 128, (dc + 1) * 128)
                    pa = psbank.tile([128, 512], f32, tag="bank")
                    for ec in range(DC):
                        nc.tensor.matmul(
                            pa[:],
                            wqT[:, ec, dsl],
                            wkT[:, ec, :],
                            start=(ec == 0),
                            stop=(ec == DC - 1),
                        )
                    nc.scalar.copy(A_r[:, dc, :], pa[:])

            with (
                tc.tile_pool(name="pTp", bufs=1) as pTp,
                tc.tile_pool(name="gtp", bufs=2) as gtp,
                tc.tile_pool(name="redp", bufs=1) as redp,
                tc.tile_pool(name="ostage", bufs=2) as ostage,
            ):
                pT = pTp.tile([128, JT, 512], f32r)

                def attention(b):
                    xT = xTs[b]
                    for ib in range(NIB):
                        isl = slice(ib * 512, (ib + 1) * 512)
                        gt = gtp.tile([128, DC, 512], f32r)
                        for ec in range(DC):
                            esl = slice(ec * 128, (ec + 1) * 128)
                            pg = psbank.tile([128, 512], f32, tag="bank")
                            for dc in range(DC):
                                nc.tensor.matmul(
                                    pg[:],
                                    A_r[:, dc, esl],
                                    xT[:, dc, isl],
                                    start=(dc == 0),
                                    stop=(dc == DC - 1),
                                )
                            nc.scalar.copy(gt[:, ec, :], pg[:])
                        for jt in range(JT):
                            jsl = slice(jt * 128, (jt + 1) * 128)
                            ps = psbank.tile([128, 512], f32, tag="bank")
                            for c in range(DC):
                                nc.tensor.matmul(
                                    ps[:],
                                    xT[:, c, jsl],
                                    gt[:, c, :],
                                    start=(c == 0),
                                    stop=(c == DC - 1),
                                )
                            nc.scalar.activation(
                                pT[:, jt, :],
                                ps[:],
                                mybir.ActivationFunctionType.Exp,
                                bias=wT[:, jt : jt + 1],
                            )
                        red = redp.tile([128, 2, 512], f32r, tag="red")
                        for g in range(2):
                            nc.vector.tensor_add(
                                red[:, g, :], pT[:, 8 * g, :], pT[:, 8 * g + 1, :]
                            )
                            for j in range(8 * g + 2, 8 * g + 8):
                                nc.vector.tensor_add(
                                    red[:, g, :], red[:, g, :], pT[:, j, :]
                                )
                        nc.vector.tensor_add(red[:, 0, :], red[:, 0, :], red[:, 1, :])
                        sums_p = psbank.tile([1, 512], f32, tag="bank")
                        nc.tensor.matmul(
                            sums_p[:], ones_r, red[:, 0, :], start=True, stop=True
                        )
                        s_sb = spool.tile([1, 512], f32)
                        nc.vector.tensor_copy(s_sb[:], sums_p[:])
                        st_p = pstrans.tile([128, 4], f32, tag="st")
                        for c in range(4):
                            nc.tensor.transpose(
                                st_p[:, c : c + 1],
                                s_sb[0:1, c * 128 : (c + 1) * 128],
                                ident[0:1, 0:1],
                            )
                        r_sb = rpool.tile([128, 4], f32, tag="r")
                        nc.vector.reciprocal(r_sb[:], st_p[:])
                        for isub in range(4):
                            po = pspv.tile([128, 512], f32)
                            for jt in range(JT):
                                nc.tensor.matmul(
                                    po[:],
                                    pT[:, jt, isub * 128 : (isub + 1) * 128],
                                    vv[:, jt, :],
                                    start=(jt == 0),
                                    stop=(jt == JT - 1),
                                )
                            ob = ostage.tile([128, 512], f32, tag="ob")
                            t0 = ib * 512 + isub * 128
                            nc.scalar.mul(ob[:], po[:], r_sb[:, isub : isub + 1])
                            nc.scalar.dma_start(out=out[b, t0 : t0 + 128, :], in_=ob[:])

                for w in range(NW):
                    w_rows(xTs[0], w)
                attention(0)

                for w in range(NW):
                    v_window(xTs[1], w)
                for w in range(NW):
                    w_rows(xTs[1], w)
                attention(1)

    nc.finalize()
    return nc


_built = None


def kernel(x, Wq, bq, Wk, bk, Wv, bv):
    global _built
    x = np.asarray(x, dtype=np.float32)
    xt = np.ascontiguousarray(np.swapaxes(x, 1, 2))  # [B, D, N]
    ws = {
        "WqT": np.ascontiguousarray(np.asarray(Wq, dtype=np.float32).T),
        "bq": np.ascontiguousarray(np.asarray(bq, dtype=np.float32)),
        "WkT": np.ascontiguousarray(np.asarray(Wk, dtype=np.float32).T),
        "Wv": np.ascontiguousarray(np.asarray(Wv, dtype=np.float32)),
        "bv": np.ascontiguousarray(np.asarray(bv, dtype=np.float32)),
    }
    if _built is None:
        _built = build()
    in_maps = [
        {"xt": np.ascontiguousarray(xt[c * PB : (c + 1) * PB]), **ws}
        for c in range(NCORES)
    ]
    res = run_bass_kernel_spmd(_built, in_maps, core_ids=list(range(NCORES)))
    kernel.last_exec_time_ns = res.exec_time_ns
    return np.concatenate([r["out"] for r in res.results], axis=0)


kernel.last_exec_time_ns = None
